# revision 76
# baseline (speedup 1.0000x reference)
"""BinLinear Trainium2 kernel: out = x @ sign(W)^T + sign(bias).

Full shapes: x [8192, 4096] f32, W [4096, 4096] f32, bias [4096] f32,
out [8192, 4096] f32.

Strategy (8 NeuronCores, data-parallel on the token dim M):
  - Each core gets x_shard = x[1024*i : 1024*(i+1)], full W, full bias and
    computes its out shard [1024, 4096]. No collectives; host concatenates.
  - Per core: x^T is made resident in SBUF ([K, M_shard] tiles, 16 MB) via a
    block-swizzled DMA load + DVE 32x32 stream-transpose (DMA transpose
    hardware is 16-bit only, fp32 needs this two-step).
  - W streams through once: swizzled DMA -> ScalarE Sign -> DVE stream
    transpose, giving binarized W^T tiles [128, 512].
  - TensorE accumulates psum[m] over 32 k-tiles in float32r (full-rate fp32
    mode: 1 cycle/row at free-dim >= 256, vs 4 cycles/row for plain fp32).
    sign(W) is exactly representable so products are exact; accumulation is
    fp32 in PSUM.
  - sign(bias) is broadcast to [128, N] once; the PSUM->SBUF eviction is a
    fused DVE tensor+tensor add of that bias.
"""

import numpy as np

import concourse.bass as bass
import concourse.mybir as mybir
import concourse.tile as tile
from concourse.vector_clock import ScopedClock, VectorClock
from concourse.tile import add_dep_helper
from concourse.bass_utils import run_bass_kernel_spmd


class SplitDrainTileContext(tile.TileContext):
    """TileContext whose kernel-tail drain is split into several drain
    instructions. The stock tail emits ONE drain waiting on every active proc
    (engines + all DMA lanes, ~15 waits) which overflows the CTRL
    instruction's sync-wait slots in walrus codegen. Emitting the same waits
    across several drains (<= 4 waits each) is semantically identical: each
    drain's waits are satisfied in turn and the final state is 'everything
    quiesced'."""

    MAX_DRAIN_WAITS = 1

    def _drain_and_barrier(self, tick_clock, wait_clock):
        gc = tick_clock.global_clock
        n = len(gc)
        for lo in range(0, n, self.MAX_DRAIN_WAITS):
            vc = VectorClock()
            for p in range(lo, min(lo + self.MAX_DRAIN_WAITS, n)):
                if gc[p]:
                    vc.require_at_least(p, gc[p])
            drain_inst = self.nc.sync.drain()
            wait_clock.add_sem_waits(
                drain_inst.ins, ScopedClock({None: vc})
            )
        self.nc.all_engine_barrier()
        assert self.sems is not None
        popped = self.nc._tile_sem_poison_stack.pop()
        assert popped is self._sem_poison
        self.nc.clear_and_free_semaphores(list(self.sems.allocated().values()))
        self.nc.all_engine_barrier()

P = 128
NFREE = 512  # moving free dim per matmul (one PSUM bank of fp32)

M_FULL, K_FULL, N_FULL = 8192, 4096, 4096
N_CORES = 8
M_SHARD = M_FULL // N_CORES


def _swizzled_load(nc, sbuf_tile, dram_ap):
    """Load dram_ap ([R, 128] slice) into sbuf_tile [128, R] block-swizzled so
    that a DVE 32x32 stream transpose of sbuf_tile yields dram_ap.T.

    Pre-DVE we need:  sbuf[32g+a, 32b+c] = dram[32b+a, 32g+c]
    so post-DVE:      out[32g+a, 32b+c] = dram[32b+c, 32g+a] = dram.T[p, f].

    DMA access patterns are limited to 3 dims, so issue one DMA per
    partition-group g (source dims [a, b, c], 128-byte contiguous runs).

    Issued from the ACT sequencer's HWDGE queue: HWDGE DMA instructions only
    accept ONE sync-wait command, and the ACT engine's vector clock has
    already observed the DVE ticks that release the destination tile slot
    (ACT waits on DVE outputs every tile), so those waits are elided and only
    the DMA-lane wait remains.
    """
    for g in range(4):
        nc.scalar.dma_start(
            sbuf_tile[32 * g : 32 * (g + 1), :],
            dram_ap[:, 32 * g : 32 * (g + 1)].rearrange("(b a) c -> a b c", a=32),
        )


def _act_claim(nc, tile_ap, src):
    """Slot-recycling helper for DVE-written tiles. The first accessor of a
    recycled pool slot inherits waits on ALL the old tile's accessor procs;
    only ACT instructions have enough sync-wait slots for that. So ACT
    'claims' the slot with a 1-element copy, then a 1-element in-place DVE
    copy (RAW on the claim) moves the ACT tick onto DVE's vector clock. The
    real DVE writer that follows then needs only its own-engine wait."""
    s = tile_ap[0:1, 0:1]
    ai = nc.scalar.activation(s, src, mybir.ActivationFunctionType.Copy)
    nc.vector.tensor_copy(out=s, in_=s)
    return ai


def _touch4(nc, sbuf_tile):
    """In-place 1-element DVE copies, one per partition group. Each waits on
    one of the 4 swizzle DMAs, advancing the DVE's observed semaphore ticks so
    the full-width consumer that follows needs no waits of its own (the HW
    allows only a few sync-wait commands per instruction)."""
    for g in range(4):
        s = sbuf_tile[32 * g : 32 * (g + 1), 0:1]
        nc.vector.tensor_copy(out=s, in_=s)


def bin_linear_tile_kernel(tc, x_ap, w_ap, b_ap, o_ap, mm_dtype=mybir.dt.bfloat16):
    """mm_dtype selects the TensorE path:
      - bfloat16: x is split into x_hi + x_lo (both bf16); two matmuls per
        tile accumulate into the same PSUM bank. sign(W) is +-1 (exact in
        bf16) so every product is exact; only the fp32 PSUM accumulation
        rounds => fp32-grade accuracy at 2 matmuls/tile.
      - float32r: single matmul per tile at the same per-matmul rate, but the
        HW rounds fp32r operands to ~12 mantissa bits => ~1e-4 rel error.
    """
    nc = tc.nc
    f32 = mybir.dt.float32
    hi_lo = mm_dtype == mybir.dt.bfloat16

    MS, K = x_ap.shape  # m per core, contraction
    N = w_ap.shape[0]
    KT = K // P  # k tiles
    MT = MS // P  # m tiles (psum banks used per n-strip)
    NS = N // NFREE  # n strips
    assert MT <= 8, "psum accumulators exceed the 8 PSUM banks"

    with (
        tc.tile_pool(name="xt", bufs=1) as xt_pool,
        tc.tile_pool(name="xswz", bufs=2) as xswz_pool,
        tc.tile_pool(name="wswz", bufs=4) as wswz_pool,
        tc.tile_pool(name="wsgn", bufs=2) as wsgn_pool,
        tc.tile_pool(name="wt", bufs=3) as wt_pool,
        tc.tile_pool(name="outp", bufs=8) as out_pool,
        tc.tile_pool(name="bias", bufs=1) as bias_pool,
        tc.tile_pool(name="psum", bufs=8, space="PSUM") as psum_pool,
    ):
        # sign(bias) striped [NS, NFREE] (partition ns holds strip ns; bf16 is
        # exact for +-1/0). It enters the output via a rank-1 (K=1) matmul
        # ones[ns]^T @ bias_sgn[ns] accumulated into each PSUM bank, so the
        # eviction is a single PSUM->DRAM DMA and matmuls keep 1-proc waits.
        bias_sgn = bias_pool.tile([1, N], mm_dtype)
        ones_row = bias_pool.tile([1, P], mm_dtype)
        claim_src = bias_pool.tile([1, 1], f32)
        nc.vector.memset(claim_src[:], 0.0)
        NBC = N // NFREE  # bias chunks
        bstg_hist = []

        def emit_bias_chunk(c):
            # Interleaved into the x loop so the bstg slot's ACT (Sign) wait
            # is well outside the ACT queue depth by reallocation time.
            bstg = xswz_pool.tile(
                [1, NFREE], f32, name=f"bstg_{c}", tag="bstg", bufs=4
            )
            bstg_hist.append(bstg)
            nc.scalar.dma_start(bstg[:], b_ap[None, c * NFREE : (c + 1) * NFREE])
            nc.scalar.activation(
                bias_sgn[:, c * NFREE : (c + 1) * NFREE],
                bstg[:],
                mybir.ActivationFunctionType.Sign,
            )

        # x^T resident: [128, KT, MS]; tile kt holds x[:, kt*128:(kt+1)*128].T
        # Allocated as mm_dtype (float32r): the DVE transpose rounds on write,
        # which the FP32r matmult verifier requires of its operand producers.
        # The fp32r matmul's LDWEIGHTS accepts only ONE sync wait, so every
        # matmul operand (and the psum slot release) must be produced on the
        # SAME engine proc (ACT): waits on one proc merge into one command.
        xt_hi = xt_pool.tile([P, KT, MS], mm_dtype, name="xt_hi")
        xt_lo = xt_pool.tile([P, KT, MS], mm_dtype, name="xt_lo") if hi_lo else None
        for kt in range(KT):
            # bufs=4: slot reuse distance = 16 DMAs = 2 full rotations of the
            # 8 HWDGE lanes, so the issuing engine's own-lane wait chain has
            # already observed every old writer lane by reallocation time and
            # the slot-allocating DMA keeps a single wait.
            xs = xswz_pool.tile([P, MS], f32, name=f"xs_{kt}", tag="xs", bufs=4)
            _swizzled_load(nc, xs, x_ap[:, kt * P : (kt + 1) * P])
            _touch4(nc, xs)
            xtr = xswz_pool.tile([P, MS], f32, name=f"xtr_{kt}", tag="xtr", bufs=1)
            nc.vector.transpose(xtr[:], xs[:])
            if not hi_lo:
                nc.scalar.activation(
                    xt_hi[:, kt, :], xtr[:], mybir.ActivationFunctionType.Copy
                )
            else:
                # hi is rounded on DVE so the x_lo subtract has all-DVE deps
                # (the TensorTensor struct takes a single sync wait); ACT then
                # re-copies hi/lo so matmuls keep a single-proc (ACT) wait.
                # The slots being recycled were last read by ACT; a 1-element
                # DVE "observer" copy (overwritten immediately, so harmless)
                # carries that ACT wait and forces ordering, leaving the real
                # op with only its own-engine wait.
                xhid = xswz_pool.tile([P, MS], mm_dtype, name=f"xhid_{kt}", tag="xhid", bufs=2)
                _act_claim(nc, xhid, claim_src[:])
                nc.vector.tensor_copy(out=xhid[:], in_=xtr[:])
                nc.scalar.activation(
                    xt_hi[:, kt, :], xhid[:], mybir.ActivationFunctionType.Copy
                )
                xlr = xswz_pool.tile([P, MS], mm_dtype, name=f"xlr_{kt}", tag="xlr", bufs=2)
                _act_claim(nc, xlr, claim_src[:])
                nc.vector.tensor_sub(out=xlr[:], in0=xtr[:], in1=xhid[:])
                nc.scalar.activation(
                    xt_lo[:, kt, :], xlr[:], mybir.ActivationFunctionType.Copy
                )
            if kt < NBC:
                emit_bias_chunk(kt)

        for c in range(min(KT, NBC), NBC):
            emit_bias_chunk(c)
        # ones = Copy(0*x + 1), produced on ACT like all matmul operands.
        nc.scalar.activation(
            ones_row[:],
            bstg_hist[0][:, 0:P],
            mybir.ActivationFunctionType.Copy,
            bias=1.0,
            scale=0.0,
        )

        # PSUM accumulators allocated ONCE: per-strip reallocation would
        # put pool-allocator waits [PE, DVE] (never own-elided) on the first
        # matmul of each bank. With fixed tiles only data deps remain: the
        # WAR on the previous strip's eviction read (DVE, 1 wait) and the
        # PE-to-PE accumulation deps, which Tile never emits waits for.
        psums = [
            psum_pool.tile([P, NFREE], f32, name=f"psum_{mi}", tag="acc")
            for mi in range(MT)
        ]
        H = NFREE // 2
        deferred_dmas = []

        def emit_out_dma(item):
            ot_, mi_, h_, nlo_ = item
            return nc.scalar.dma_start(
                o_ap[
                    mi_ * P : (mi_ + 1) * P,
                    nlo_ + h_ * H : nlo_ + (h_ + 1) * H,
                ],
                ot_[:],
            )
        for ns in range(NS):
            n_lo = ns * NFREE
            # bias enters PSUM first: rank-1 matmul, start=True clears banks.
            for mi in range(MT):
                nc.tensor.matmul(
                    psums[mi][:],
                    ones_row[:],
                    bias_sgn[:, n_lo : n_lo + NFREE],
                    start=True,
                    stop=False,
                )
            for kt in range(KT):
                wsz = wswz_pool.tile([P, NFREE], f32)
                _swizzled_load(nc, wsz, w_ap[n_lo : n_lo + NFREE, kt * P : (kt + 1) * P])
                _touch4(nc, wsz)
                wtr = wsgn_pool.tile([P, NFREE], f32)
                _act_claim(nc, wtr, claim_src[:])
                if kt == 2 and deferred_dmas:
                    # previous strip's out-DMAs, order-pinned behind its
                    # eviction claim: ACT's clock covers the copies, so each
                    # DMA elides its DVE data wait and keeps the lane wait.
                    for item in deferred_dmas:
                        di = emit_out_dma(item)
                        add_dep_helper(di.ins, last_eclaim.ins, sync=False,
                                       reason="deferred out dma after eclaim")
                    deferred_dmas = []
                nc.vector.transpose(wtr[:], wsz[:])
                wtt = wt_pool.tile([P, NFREE], mm_dtype, bufs=4)
                nc.scalar.activation(wtt[:], wtr[:], mybir.ActivationFunctionType.Sign)
                rhs = wtt[:]
                last = kt == KT - 1
                for mi in range(MT):
                    nc.tensor.matmul(
                        psums[mi][:],
                        xt_hi[:, kt, mi * P : (mi + 1) * P],
                        rhs,
                        start=False,
                        stop=(last and not hi_lo),
                    )
                    if hi_lo:
                        nc.tensor.matmul(
                            psums[mi][:],
                            xt_lo[:, kt, mi * P : (mi + 1) * P],
                            rhs,
                            start=False,
                            stop=last,
                        )
                # Lagged PE observation on ACT: an in-place 1-element copy of
                # an lhsT element the matmuls of 2 tiles ago read. It waits
                # [PE >= those matmuls] (already done - no stall) and lets the
                # Sign 2 tiles later elide its wtt-slot-release PE wait.
                if kt >= 2 or ns > 0:
                    pkt = kt - 2 if kt >= 2 else KT + kt - 2
                    s = xt_hi[0:1, pkt, 0:1]
                    nc.scalar.activation(s, s, mybir.ActivationFunctionType.Copy)
            # One in-place DVE touch of the LAST bank's first element: it
            # waits for the final stop-matmul of the strip, putting PE on
            # DVE's clock so every eviction copy below elides its PE wait.
            s = psums[MT - 1][0:1, 0:1]
            pe_touch = nc.vector.tensor_copy(out=s, in_=s)
            # Evict in [128, 256] halves: 16 copies/strip across 8 slots, so
            # a recycled slot's previous DVE writer is >= 8 instructions back
            # (same-engine waits within the queue depth would be emitted and
            # blow the 1-wait budget). Each copy then carries only the DMASW
            # slot-release wait.



            for j in range(2 * MT):
                mi, h = divmod(j, 2)
                # 16 slots: no within-strip recycling; the across-strip
                # allocator wait is just the old reader's DMASW lane tick.
                ot = out_pool.tile(
                    [P, H], f32, name=f"ot_{ns}_{mi}_{h}", tag="ot", bufs=16
                )
                cpi = nc.vector.tensor_copy(
                    out=ot[:], in_=psums[mi][:, h * H : (h + 1) * H]
                )
                # order-only edge: copy runs after the PE-observing touch so
                # its PE data wait is elided (single DMASW slot wait remains)
                add_dep_helper(cpi.ins, pe_touch.ins, sync=False,
                               reason="evac copy after PE-observing touch")
                deferred_dmas.append((ot, mi, h, n_lo))
            # ACT observes this strip's last eviction copy (hence all 16:
            # same DVE proc, monotone ticks). The deferred out-DMAs pinned
            # after this claim elide their DVE data wait deterministically.
            ecl = bias_pool.tile([1, 1], f32, name=f"ecl_{ns}", tag="ecl", bufs=2)
            last_eclaim = _act_claim(nc, ecl, deferred_dmas[-1][0][0:1, 0:1])

        for item in deferred_dmas:
            di = emit_out_dma(item)
            add_dep_helper(di.ins, last_eclaim.ins, sync=False,
                           reason="final deferred out dma")


def build_module(m_shard=M_SHARD, k=K_FULL, n=N_FULL, mm_dtype=mybir.dt.bfloat16):
    nc = bass.Bass("TRN2", target_bir_lowering=False, debug=False)
    f32 = mybir.dt.float32
    x_d = nc.dram_tensor("x", [m_shard, k], f32, kind="ExternalInput")
    w_d = nc.dram_tensor("weight", [n, k], f32, kind="ExternalInput")
    b_d = nc.dram_tensor("bias", [n], f32, kind="ExternalInput")
    o_d = nc.dram_tensor("out", [m_shard, n], f32, kind="ExternalOutput")
    with SplitDrainTileContext(nc) as tc:
        bin_linear_tile_kernel(tc, x_d.ap(), w_d.ap(), b_d.ap(), o_d.ap(), mm_dtype)
    return nc


_NC_CACHE = {}


def _get_module():
    if "nc" not in _NC_CACHE:
        _NC_CACHE["nc"] = build_module()
    return _NC_CACHE["nc"]


def make_in_maps(x, weight, bias):
    x = np.ascontiguousarray(np.asarray(x, dtype=np.float32))
    weight = np.ascontiguousarray(np.asarray(weight, dtype=np.float32))
    bias = np.ascontiguousarray(np.asarray(bias, dtype=np.float32))
    return [
        {
            "x": x[i * M_SHARD : (i + 1) * M_SHARD],
            "weight": weight,
            "bias": bias,
        }
        for i in range(N_CORES)
    ]


def gather(results):
    return np.concatenate([results[i]["out"] for i in range(N_CORES)], axis=0)


def run(x, weight, bias, trace=False, **kw):
    """Run on the 8 NeuronCores; returns (out_full, BassKernelResults)."""
    nc = _get_module()
    in_maps = make_in_maps(x, weight, bias)
    res = run_bass_kernel_spmd(nc, in_maps, list(range(N_CORES)), trace=trace, **kw)
    return gather(res.results), res


def kernel(x, weight, bias):
    out, _ = run(x, weight, bias)
    return out
